# revision 7
# baseline (speedup 1.0000x reference)
"""Trainium2 Bass kernel: sparse (sliding-window) attention block.

Full module per reference:
  RMSNorm -> fused QKV (5120x2880) -> YaRN RoPE -> GQA sliding-window(128)
  causal attention with learned sink logit -> out projection (2880x4096).

Sharding: tensor-parallel over heads across 8 cores. Core c owns q-heads
[8c, 8c+8) and kv-head c. RMSNorm is computed (replicated) on every core.
Each core emits a partial [1024, 2880] output in bf16 (its heads' out-proj
contribution); the host sums the 8 partials in f64 and adds out_b.

V2 design (token-major QKV): the QKV matmul keeps x k-tiles stationary and
streams the weights, so the projection comes out token-major [128 tok, 640]:
  - the RMSNorm rescale becomes a per-partition scalar -> fused with the
    bias add into one scalar_tensor_tensor drain per token tile
  - RoPE pairs live on the free axis -> pure strided-AP muls, no partition
    swap copies
  - v needs no transpose for AV; q/k are transposed on the PE
Attention for token tile b runs right after its QKV chain, interleaved with
the remaining QKV chains, so the PE never idles (keeps HAM at full clock).

Per-core DRAM inputs (host-prearranged, all contiguous DMAs):
  xt    [23, 128, 1024] bf16  x.T k-tiles (zero-padded last tile)
  wk    [23, 128, 640] bf16   qkv weights k-major (cols: 512 q | 64 v | 64 k),
                              pre-scaled by norm_scale
  wout  [4, 128, 2880] bf16   out_w.T shard rhs tiles (hd k-tile, hd-part, H)
  bias  [128, 640] bf16       qkv bias, broadcast along partitions
  cosq/sinq/cosk/sink_t [128, 8, 64] bf16  rope tables token-major
                              (q tables pre-scaled by sm_scale, sin signed)
  mask  [128, 256] f32        additive {0,-1e30}: [prev-tile | self-tile]
  esink [128, 8] f32          exp(sinks) broadcast to 128 partitions
Output: y [1024, 2880] bf16 partial.
"""

import math
import sys

import numpy as np

try:
    import concourse.bass as bass
except ImportError:  # pragma: no cover
    sys.path.insert(0, "/opt/trn_rl_repo")
    import concourse.bass as bass

import concourse.bacc as bacc
import concourse.tile as tile
from concourse import mybir
from concourse.masks import make_identity
from concourse.bass_utils import run_bass_kernel_spmd

import ml_dtypes

BF16 = ml_dtypes.bfloat16

T = 1024
HIDDEN = 2880
HD = 64
NH = 64
NKV = 8
SW = 128
NCORES = 8
HPC = NH // NCORES          # q heads per core = 8
QKV_DIM = HD * (NH + 2 * NKV)
SM_SCALE = 1.0 / math.sqrt(HD)

P = 128
KT = 23                      # k-tiles over hidden (zero-padded to 23*128)
NQ = HPC * HD                # 512 q columns per core
NC = NQ + 2 * HD             # 640 qkv columns per core (q | v | k)
MT = T // P                  # 8 token tiles
NEG = -1.0e30
AW = HD + 1                  # per-head AV width (64 v-dims + denominator)
YC = 480                     # out-proj psum chunk width (6 chunks of 480)

dt = mybir.dt
AF = mybir.ActivationFunctionType
OP = mybir.AluOpType

_CACHE = {}


# ----------------------------------------------------------------------------
# host-side helpers
# ----------------------------------------------------------------------------

def _rope_cos_sin(num_tokens):
    base = 150000.0
    scaling = 32.0
    init_ctx = 4096.0
    ntk_alpha = 1.0
    ntk_beta = 32.0
    d_half = HD / 2
    freq = base ** (np.arange(0, HD, 2, dtype=np.float32) / HD)
    concentration = 0.1 * math.log(scaling) + 1.0
    low = d_half * math.log(init_ctx / (ntk_beta * 2 * math.pi)) / math.log(base)
    high = d_half * math.log(init_ctx / (ntk_alpha * 2 * math.pi)) / math.log(base)
    interpolation = 1.0 / (scaling * freq)
    extrapolation = 1.0 / freq
    ramp = (np.arange(int(d_half), dtype=np.float32) - low) / (high - low)
    m = 1.0 - np.clip(ramp, 0.0, 1.0)
    inv_freq = interpolation * (1.0 - m) + extrapolation * m
    t = np.arange(num_tokens, dtype=np.float32)
    freqs = t[:, None] * inv_freq[None, :]
    cos = (np.cos(freqs) * concentration).astype(np.float32)
    sin = (np.sin(freqs) * concentration).astype(np.float32)
    return cos, sin  # [T, 32]


def _host_masks():
    j = np.arange(P)[:, None]   # kt row (partition)
    i = np.arange(P)[None, :]   # q col (free)
    mask_prev = np.where(j > i, 0.0, NEG).astype(np.float32)   # dist in [1,127]
    mask_self = np.where(j <= i, 0.0, NEG).astype(np.float32)  # dist in [0,127]
    return np.concatenate([mask_prev, mask_self], axis=1)  # [128, 256]


def _rope_tables():
    cos, sin = _rope_cos_sin(T)  # [1024, 32]
    cos64 = np.concatenate([cos, cos], axis=1)             # [1024, 64]
    sin64 = np.concatenate([-sin, sin], axis=1)            # signed for halves

    def tok_major(a):  # [1024, 64] -> [128, 8, 64]
        return a.reshape(MT, P, HD).transpose(1, 0, 2).copy()

    return (
        tok_major((cos64 * SM_SCALE).astype(np.float32)).astype(BF16),
        tok_major((sin64 * SM_SCALE).astype(np.float32)).astype(BF16),
        tok_major(cos64).astype(BF16),
        tok_major(sin64).astype(BF16),
    )


def _prep_core_inputs(core, x, norm_scale, qkv_w, qkv_b, out_w, sinks):
    """Build the per-core input map (all numpy, layouts per module docstring)."""
    q_end = NH * HD
    k_end = q_end + NKV * HD

    # rows of qkv_w for this core: 8 q heads + 1 v head + 1 k head = 640 rows
    qrows = np.arange(core * HPC * HD, (core + 1) * HPC * HD)
    krows = np.arange(q_end + core * HD, q_end + (core + 1) * HD)
    vrows = np.arange(k_end + core * HD, k_end + (core + 1) * HD)
    rows = np.concatenate([qrows, vrows, krows])  # [640]: q | v | k

    wshard = (qkv_w[rows, :] * norm_scale[None, :]).astype(np.float32)  # [640, 2880]
    bshard = qkv_b[rows].astype(np.float32)  # [640]

    # k-major moving tiles: wk[ki, kp, n] = wshard[n, ki*128 + kp], zero-padded
    wk = np.zeros((KT, P, NC), dtype=BF16)
    wkt = wshard.T  # [2880, 640]
    for ki in range(KT):
        k0 = ki * P
        ksz = min(P, HIDDEN - k0)
        wk[ki, :ksz, :] = wkt[k0:k0 + ksz, :].astype(BF16)

    # x transposed k-tiles, zero-padded
    xt = np.zeros((KT, P, T), dtype=BF16)
    xT = x.T  # [2880, 1024]
    for ki in range(KT):
        k0 = ki * P
        ksz = min(P, HIDDEN - k0)
        xt[ki, :ksz, :] = xT[k0:k0 + ksz, :].astype(BF16)

    # out_w shard: columns for this core's heads -> [2880, 512] -> T -> [512, 2880]
    cols = np.arange(core * HPC * HD, (core + 1) * HPC * HD)
    wo = out_w[:, cols].T.astype(np.float32)  # [512 hd, 2880 H]
    wout = wo.reshape(4, P, HIDDEN).astype(BF16)

    bias = np.broadcast_to(bshard.astype(BF16), (P, NC)).copy()

    cosq, sinq, cosk, sink_t = _rope_tables()

    esink = np.exp(sinks[core * HPC:(core + 1) * HPC].astype(np.float64))
    esink = np.broadcast_to(esink.astype(np.float32), (P, HPC)).copy()

    return {
        "xt": xt,
        "wk": wk,
        "wout": wout,
        "bias": bias,
        "cosq": cosq, "sinq": sinq, "cosk": cosk, "sink_t": sink_t,
        "esink": esink,
    }


# ----------------------------------------------------------------------------
# device kernel (Tile)
# ----------------------------------------------------------------------------

def build_nc():
    nc = bacc.Bacc("TRN2", target_bir_lowering=False, debug=False)

    xt_d = nc.dram_tensor("xt", [KT, P, T], dt.bfloat16, kind="ExternalInput").ap()
    wk_d = nc.dram_tensor("wk", [KT, P, NC], dt.bfloat16, kind="ExternalInput").ap()
    wout_d = nc.dram_tensor("wout", [4, P, HIDDEN], dt.bfloat16, kind="ExternalInput").ap()
    bias_d = nc.dram_tensor("bias", [P, NC], dt.bfloat16, kind="ExternalInput").ap()
    cosq_d = nc.dram_tensor("cosq", [P, MT, HD], dt.bfloat16, kind="ExternalInput").ap()
    sinq_d = nc.dram_tensor("sinq", [P, MT, HD], dt.bfloat16, kind="ExternalInput").ap()
    cosk_d = nc.dram_tensor("cosk", [P, MT, HD], dt.bfloat16, kind="ExternalInput").ap()
    sink_d = nc.dram_tensor("sink_t", [P, MT, HD], dt.bfloat16, kind="ExternalInput").ap()
    esink_d = nc.dram_tensor("esink", [P, HPC], dt.float32, kind="ExternalInput").ap()
    y_d = nc.dram_tensor("y", [T, HIDDEN], dt.bfloat16, kind="ExternalOutput").ap()

    def bcast_mid(ap2d, n):
        """[P, F] -> [P, n, F] with a 0-step middle dim (free broadcast)."""
        return bass.AP(tensor=ap2d.tensor, offset=ap2d.offset,
                       ap=[ap2d.ap[0], [0, n]] + list(ap2d.ap[1:]))

    with tile.TileContext(nc) as tc:
        with (
            tc.tile_pool(name="const", bufs=1) as const,
            tc.tile_pool(name="res", bufs=1) as res,
            tc.tile_pool(name="xsqp", bufs=3) as xsqp,
            tc.tile_pool(name="qkvp", bufs=3) as qkvp,
            tc.tile_pool(name="ropep", bufs=2) as ropep,
            tc.tile_pool(name="qrap", bufs=3) as qrap,
            tc.tile_pool(name="ptp", bufs=4) as ptp,
            tc.tile_pool(name="anormp", bufs=3) as anormp,
            tc.tile_pool(name="atp", bufs=6) as atp,
            tc.tile_pool(name="smallp", bufs=4) as smallp,
            tc.tile_pool(name="ysbp", bufs=3) as ysbp,
            tc.tile_pool(name="pmix", bufs=2, space="PSUM") as pmix,
            tc.tile_pool(name="pq", bufs=4, space="PSUM") as pq,
            tc.tile_pool(name="pkvc", bufs=2, space="PSUM") as pkvc,
        ):
            # ---- constants ----
            ones_f = const.tile([P, 1], dt.float32, tag="ones", name="ones")
            nc.vector.memset(ones_f, 1.0)
            ident_b = const.tile([P, P], dt.bfloat16, tag="identb", name="identb")
            make_identity(nc, ident_b)
            ident_f = const.tile([1, 1], dt.float32, tag="identf", name="identf")
            nc.vector.memset(ident_f, 1.0)
            zbias = const.tile([P, 1], dt.float32, tag="zbias", name="zbias")
            nc.vector.memset(zbias, 0.0)
            eps_t = const.tile([P, 1], dt.float32, tag="eps", name="eps")
            nc.vector.memset(eps_t, 1e-5)

            # ---- DMA issue order: first k-tiles, then small consts, rest ----
            xt_sb = res.tile([P, KT, T], dt.bfloat16, tag="xt", name="xt")
            wk_sb = res.tile([P, KT, NC], dt.bfloat16, tag="wk", name="wk")
            for ki in range(4):
                nc.sync.dma_start(out=wk_sb[:, ki, :], in_=wk_d[ki])
                nc.sync.dma_start(out=xt_sb[:, ki, :], in_=xt_d[ki])
            bias_sb = const.tile([P, NC], dt.bfloat16, tag="bias", name="bias")
            nc.sync.dma_start(out=bias_sb, in_=bias_d)
            tabs = {}
            for nm, d in (("cosq", cosq_d), ("sinq", sinq_d),
                          ("cosk", cosk_d), ("sink_t", sink_d)):
                tabs[nm] = const.tile([P, MT, HD], dt.bfloat16, tag=nm, name=nm)
                nc.sync.dma_start(out=tabs[nm], in_=d)
            esink_sb = const.tile([P, HPC], dt.float32, tag="esink", name="esink")
            nc.sync.dma_start(out=esink_sb, in_=esink_d)
            for ki in range(4, KT):
                nc.sync.dma_start(out=wk_sb[:, ki, :], in_=wk_d[ki])
                nc.sync.dma_start(out=xt_sb[:, ki, :], in_=xt_d[ki])
            wout_sb = []
            for kk in range(4):
                w = res.tile([P, HIDDEN], dt.bfloat16, tag=f"wout{kk}",
                             name=f"wout{kk}")
                nc.sync.dma_start(out=w, in_=wout_d[kk])
                wout_sb.append(w)

            # ---- wave A: QKV chains for token tiles 0-3 chase the DMA;
            #      squares+accumulate for RMSNorm ssq ride along on DVE/GpSimd
            acc = res.tile([P, T], dt.float32, tag="acc", name="acc")
            pq_t = {}
            for t in range(4):
                pq_t[t] = pq.tile([P, NQ], dt.float32, tag="pq", name=f"pq{t}")
            for ki in range(KT):
                eng = nc.vector if ki % 2 == 0 else nc.gpsimd
                if ki == 0:
                    eng.tensor_tensor(acc, xt_sb[:, 0, :], xt_sb[:, 0, :],
                                      OP.mult)
                else:
                    xsq = xsqp.tile([P, T], dt.bfloat16, tag="xsq", name="xsq")
                    eng.tensor_tensor(xsq, xt_sb[:, ki, :], xt_sb[:, ki, :],
                                      OP.mult)
                    nc.vector.tensor_tensor(acc, acc, xsq, OP.add)
                for t in range(4):
                    lhs = xt_sb[:, ki, t * P:(t + 1) * P]
                    nc.tensor.matmul(pq_t[t], lhs, wk_sb[:, ki, 0:NQ],
                                     start=(ki == 0), stop=(ki == KT - 1))

            # ---- ssq -> rsq8 [128 tok, 8 tile] ----
            pssq = [pmix.tile([1, 512], dt.float32, tag="mix", name=f"ssq{i}")
                    for i in range(2)]
            for i in range(2):
                nc.tensor.matmul(pssq[i], ones_f, acc[:, i * 512:(i + 1) * 512],
                                 start=True, stop=True)
            ssq_sb = res.tile([1, T], dt.float32, tag="ssq", name="ssq")
            for i in range(2):
                nc.vector.tensor_copy(ssq_sb[:, i * 512:(i + 1) * 512], pssq[i])
            ssqT = pmix.tile([P, MT], dt.float32, tag="mix", name="ssqT")
            for t in range(MT):
                nc.tensor.transpose(ssqT[:, t:t + 1],
                                    ssq_sb[0:1, t * P:(t + 1) * P],
                                    ident_f)
            lnm = res.tile([P, MT], dt.float32, tag="lnm", name="lnm")
            nc.scalar.activation(lnm, ssqT, AF.Ln, bias=eps_t, scale=1.0 / HIDDEN)
            rsq8 = res.tile([P, MT], dt.float32, tag="rsq8", name="rsq8")
            nc.scalar.activation(rsq8, lnm, AF.Exp, bias=zbias, scale=-0.5)

            # ---- per-token-tile post-QKV: drain, rope, transposes ----
            krope = res.tile([HD, T], dt.bfloat16, tag="krope", name="krope")
            vtok = [res.tile([P, AW], dt.bfloat16, tag=f"vtok{b}",
                             name=f"vtok{b}")
                    for b in range(MT)]
            qra = {}

            def drain_rope(t, pq_tile, pkv_ap):
                """psum -> qkv_sb (fused rsq scale + bias); rope; PE transposes."""
                rsq = rsq8[:, t:t + 1]
                qkv_sb = qkvp.tile([P, NC], dt.bfloat16, tag="qkv", name="qkv")
                nc.vector.scalar_tensor_tensor(qkv_sb[:, 0:NQ], pq_tile, rsq,
                                               bias_sb[:, 0:NQ], OP.mult,
                                               OP.add)
                nc.vector.scalar_tensor_tensor(qkv_sb[:, NQ:NC], pkv_ap, rsq,
                                               bias_sb[:, NQ:NC], OP.mult,
                                               OP.add)
                # v -> vtok (no rope)
                nc.gpsimd.tensor_copy(vtok[t][:, 0:HD], qkv_sb[:, NQ:NQ + HD])
                nc.gpsimd.memset(vtok[t][:, HD:HD + 1], 1.0)
                # rope q: view [P, 8, 64]
                q3 = qkv_sb[:, 0:NQ].rearrange("p (h d) -> p h d", h=HPC)
                m1 = ropep.tile([P, HPC, HD], dt.bfloat16, tag="m1", name="m1")
                m2 = ropep.tile([P, HPC, HD], dt.bfloat16, tag="m2", name="m2")
                cq = tabs["cosq"][:, t, :]
                sq = tabs["sinq"][:, t, :]
                nc.vector.tensor_tensor(m1, q3, bcast_mid(cq, HPC), OP.mult)
                nc.vector.tensor_tensor(m2[:, :, 0:32], q3[:, :, 32:64],
                                        bcast_mid(sq[:, 0:32], HPC), OP.mult)
                nc.vector.tensor_tensor(m2[:, :, 32:64], q3[:, :, 0:32],
                                        bcast_mid(sq[:, 32:64], HPC), OP.mult)
                qrot = ropep.tile([P, HPC, HD], dt.bfloat16, tag="qrot",
                                  name="qrot")
                nc.vector.tensor_tensor(qrot, m1, m2, OP.add)
                # rope k: [P, 1, 64] at cols 576:640
                k3 = qkv_sb[:, NQ + HD:NC]
                ck = tabs["cosk"][:, t, :]
                sk = tabs["sink_t"][:, t, :]
                km1 = ropep.tile([P, HD], dt.bfloat16, tag="km1", name="km1")
                km2 = ropep.tile([P, HD], dt.bfloat16, tag="km2", name="km2")
                nc.gpsimd.tensor_tensor(km1, k3, ck, OP.mult)
                nc.gpsimd.tensor_tensor(km2[:, 0:32], k3[:, 32:64],
                                        sk[:, 0:32], OP.mult)
                nc.gpsimd.tensor_tensor(km2[:, 32:64], k3[:, 0:32],
                                        sk[:, 32:64], OP.mult)
                krot = ropep.tile([P, HD], dt.bfloat16, tag="krot", name="krot")
                nc.gpsimd.tensor_tensor(krot, km1, km2, OP.add)
                # PE transposes: q -> [2h*64, 128 tok] pairs; k -> [64, 128]
                qr2 = qrot.rearrange("p h d -> p (h d)")
                ptq = pmix.tile([P, 4, P], dt.bfloat16, tag="mix", name="ptq")
                for hp in range(4):
                    nc.tensor.transpose(ptq[:, hp, :],
                                        qr2[:, hp * P:(hp + 1) * P], ident_b)
                ptk = pmix.tile([HD, P], dt.bfloat16, tag="mix", name="ptk")
                nc.tensor.transpose(ptk, krot, ident_b)
                qra_t = qrap.tile([HD, HPC, P], dt.bfloat16, tag="qra",
                                  name="qra")
                # even heads from psum rows 0:64, odd heads from rows 64:128
                nc.vector.tensor_copy(
                    qra_t.rearrange("p (a b) t -> p a b t", b=2)[:, :, 0, :],
                    ptq[0:HD, :, :])
                nc.vector.tensor_copy(
                    qra_t.rearrange("p (a b) t -> p a b t", b=2)[:, :, 1, :],
                    ptq[HD:P, :, :])
                nc.scalar.copy(krope[:, t * P:(t + 1) * P], ptk)
                qra[t] = qra_t

            # ---- attention for token tile b (needs tiles b-1, b done) ----
            def attention(b):
                pts = []
                for kt, is_self in ((b - 1, False), (b, True)):
                    if kt < 0:
                        pts.append(None)
                        continue
                    pt_g = []
                    for g in range(2):
                        ps = pq.tile([P, 4, P], dt.float32, tag="pq",
                                     name="score")
                        nc.tensor.matmul(
                            ps, krope[:, kt * P:(kt + 1) * P],
                            qra[b][:, 4 * g:4 * g + 4, :],
                            start=True, stop=True)
                        pt = ptp.tile([P, 4, P], dt.bfloat16, tag="pt",
                                      name="pt")
                        nc.scalar.activation(pt, ps, AF.Exp, bias=zbias)
                        # zero masked entries: keep kv<=q (self) / kv>q (prev)
                        if is_self:
                            nc.gpsimd.affine_select(
                                out=pt, in_=pt, compare_op=OP.is_ge, fill=0.0,
                                base=0, channel_multiplier=-1,
                                pattern=[[0, 4], [1, P]])
                        else:
                            nc.gpsimd.affine_select(
                                out=pt, in_=pt, compare_op=OP.is_gt, fill=0.0,
                                base=0, channel_multiplier=1,
                                pattern=[[0, 4], [-1, P]])
                        pt_g.append(pt)
                    pts.append(pt_g)
                ptA, ptB = pts

                rec8 = smallp.tile([P, HPC], dt.float32, tag="rec8",
                                   name="rec8")
                att = []
                for g in range(2):
                    pg = pmix.tile([P, 4, AW], dt.float32, tag="mix", name="pg")
                    for j in range(4):
                        if b > 0:
                            nc.tensor.matmul(pg[:, j, :], ptA[g][:, j, :],
                                             vtok[b - 1], start=True,
                                             stop=False)
                            nc.tensor.matmul(pg[:, j, :], ptB[g][:, j, :],
                                             vtok[b], start=False, stop=True)
                        else:
                            nc.tensor.matmul(pg[:, j, :], ptB[g][:, j, :],
                                             vtok[b], start=True, stop=True)
                    g0 = 4 * g
                    nc.vector.tensor_tensor(rec8[:, g0:g0 + 4],
                                            pg[:, :, HD:HD + 1],
                                            esink_sb[:, g0:g0 + 4], OP.add)
                    nc.vector.reciprocal(rec8[:, g0:g0 + 4], rec8[:, g0:g0 + 4])
                    an = anormp.tile([P, 4, HD], dt.bfloat16, tag="anorm",
                                     name="anorm")
                    rec3 = bass.AP(tensor=rec8.tensor,
                                   offset=rec8[:, g0:g0 + 4].offset,
                                   ap=[rec8.ap[0], [1, 4], [0, HD]])
                    nc.vector.tensor_tensor(an, pg[:, :, 0:HD], rec3, OP.mult)
                    a2 = an.rearrange("p a b -> p (a b)")
                    for j in range(2):
                        pat = pmix.tile([P, P], dt.bfloat16, tag="mix",
                                        name="pat")
                        nc.tensor.transpose(pat, a2[:, j * P:(j + 1) * P],
                                            ident_b)
                        at = atp.tile([P, P], dt.bfloat16, tag="at", name="at")
                        if (g + j) % 2 == 0:
                            nc.vector.tensor_copy(at, pat)
                        else:
                            nc.scalar.copy(at, pat)
                        att.append(at)

                # out projection in waves of 2 chunks (shares att LDW)
                for w0 in range(0, HIDDEN // YC, 2):
                    pys = [pq.tile([P, YC], dt.float32, tag="pq", name="py")
                           for _ in range(2)]
                    for kk in range(4):
                        for ci in range(2):
                            o0 = (w0 + ci) * YC
                            nc.tensor.matmul(pys[ci], att[kk],
                                             wout_sb[kk][:, o0:o0 + YC],
                                             start=(kk == 0), stop=(kk == 3))
                    for ci in range(2):
                        o0 = (w0 + ci) * YC
                        ysb = ysbp.tile([P, YC], dt.bfloat16, tag="ysb",
                                        name="ysb")
                        nc.scalar.activation(ysb, pys[ci], AF.Copy)
                        nc.sync.dma_start(out=y_d[b * P:(b + 1) * P,
                                                  o0:o0 + YC],
                                          in_=ysb)

            # kv chains: short ki-inner accumulations (one pending group
            # per 2KB psum region), run after xt/wk are resident
            def kv_chain(t):
                kvt = pkvc.tile([P, NQ], dt.float32, tag="kvc", name=f"kv{t}")
                for ki in range(KT):
                    nc.tensor.matmul(kvt[:, 0:2 * HD],
                                     xt_sb[:, ki, t * P:(t + 1) * P],
                                     wk_sb[:, ki, NQ:NC],
                                     start=(ki == 0), stop=(ki == KT - 1))
                return kvt

            def q_chain(t):
                pq_tile = pq.tile([P, NQ], dt.float32, tag="pq", name=f"pq{t}")
                for ki in range(KT):
                    nc.tensor.matmul(pq_tile, xt_sb[:, ki, t * P:(t + 1) * P],
                                     wk_sb[:, ki, 0:NQ],
                                     start=(ki == 0), stop=(ki == KT - 1))
                return pq_tile

            # wave A drains (tiles 0-3) interleaved with their kv chains
            kv0 = kv_chain(0)
            kv1 = kv_chain(1)
            drain_rope(0, pq_t[0], kv0[:, 0:2 * HD])
            kv2 = kv_chain(2)
            drain_rope(1, pq_t[1], kv1[:, 0:2 * HD])
            kv3 = kv_chain(3)
            drain_rope(2, pq_t[2], kv2[:, 0:2 * HD])
            drain_rope(3, pq_t[3], kv3[:, 0:2 * HD])

            # wave B chains interleaved with attention
            for t in range(4, MT):
                pq_tile = q_chain(t)
                kvt = kv_chain(t)
                drain_rope(t, pq_tile, kvt[:, 0:2 * HD])
                attention(t - 4)
            for b in range(4, MT):
                attention(b)

    nc.compile()
    return nc


# ----------------------------------------------------------------------------
# public entry
# ----------------------------------------------------------------------------

LAST_RESULTS = None


def kernel(x, norm_scale, qkv_w, qkv_b, out_w, out_b, sinks):
    global LAST_RESULTS
    x = np.asarray(x, dtype=np.float32)
    norm_scale = np.asarray(norm_scale, dtype=np.float32)
    qkv_w = np.asarray(qkv_w, dtype=np.float32)
    qkv_b = np.asarray(qkv_b, dtype=np.float32)
    out_w = np.asarray(out_w, dtype=np.float32)
    out_b = np.asarray(out_b, dtype=np.float32)
    sinks = np.asarray(sinks, dtype=np.float32)

    if "nc" not in _CACHE:
        _CACHE["nc"] = build_nc()
    nc = _CACHE["nc"]

    in_maps = [
        _prep_core_inputs(c, x, norm_scale, qkv_w, qkv_b, out_w, sinks)
        for c in range(NCORES)
    ]
    import os
    tmpdir = os.environ.get("BASS_TMPDIR") or None
    res = run_bass_kernel_spmd(nc, in_maps, core_ids=list(range(NCORES)),
                               tmpdir=tmpdir)
    LAST_RESULTS = res
    y = np.zeros((T, HIDDEN), dtype=np.float64)
    for c in range(NCORES):
        y += res.results[c]["y"].astype(np.float64)
    y += out_b.astype(np.float64)[None, :]
    return y.astype(np.float32)


# revision 8
# speedup vs baseline: 1.3410x; 1.3410x over previous
"""Trainium2 Bass kernel: sparse (sliding-window) attention block.

Full module per reference:
  RMSNorm -> fused QKV (5120x2880) -> YaRN RoPE -> GQA sliding-window(128)
  causal attention with learned sink logit -> out projection (2880x4096).

Sharding: tensor-parallel over heads across 8 cores. Core c owns q-heads
[8c, 8c+8) and kv-head c. RMSNorm is computed (replicated) on every core.
Each core emits a partial [1024, 2880] output in bf16 (its heads' out-proj
contribution); the host sums the 8 partials in f64 and adds out_b.

V3 design (token-major QKV):
  - QKV keeps x k-tiles stationary, streams weights -> token-major [tok, 640]
    output; RMSNorm rescale is a per-partition scalar fused with the bias add
    (one scalar_tensor_tensor drain); RoPE pairs live on the free axis
    (strided-AP muls, no partition-swap copies); v feeds AV untransposed.
  - ssq for RMSNorm rides the DMA-chase phase as PE ones-matmuls.
  - Attention for tile b runs right after its QKV chain; out-proj is
    software-pipelined one tile behind so the PE never waits on exp/mask.
  - All DMAs are consolidated (each DMA has ~600ns fixed cost): 4 xt groups,
    4 wk groups, one const blob, one wout, one y write per token tile.

Per-core DRAM inputs (host-prearranged, partition-major, contiguous):
  xt    [128, 23, 1024] bf16  x.T k-tiles (zero-padded last tile)
  wk    [128, 23, 640] bf16   qkv weights k-major (cols: 512 q | 64 v | 64 k),
                              pre-scaled by norm_scale
  wout  [128, 4, 2880] bf16   out_w.T shard rhs tiles
  cblob [128, 2952] bf16      bias(640) | mask01(256) | esink(8) |
                              cosq,sinq,cosk,sinkt (4x512, token-major,
                              q tables pre-scaled by sm_scale, sin signed)
Output: y [1024, 2880] bf16 partial.
"""

import math
import sys

import numpy as np

try:
    import concourse.bass as bass
except ImportError:  # pragma: no cover
    sys.path.insert(0, "/opt/trn_rl_repo")
    import concourse.bass as bass

import concourse.bacc as bacc
import concourse.tile as tile
from concourse import mybir
from concourse.masks import make_identity
from concourse.bass_utils import run_bass_kernel_spmd

import ml_dtypes

BF16 = ml_dtypes.bfloat16

T = 1024
HIDDEN = 2880
HD = 64
NH = 64
NKV = 8
SW = 128
NCORES = 8
HPC = NH // NCORES          # q heads per core = 8
QKV_DIM = HD * (NH + 2 * NKV)
SM_SCALE = 1.0 / math.sqrt(HD)

P = 128
KT = 23                      # k-tiles over hidden (zero-padded to 23*128)
NQ = HPC * HD                # 512 q columns per core
NC = NQ + 2 * HD             # 640 qkv columns per core (q | v | k)
MT = T // P                  # 8 token tiles
AW = HD + 1                  # per-head AV width (64 v-dims + denominator)
YC = 480                     # out-proj psum chunk width (6 chunks of 480)
CB = NC + 2 * P + HPC + 4 * MT * HD   # const blob cols = 2952

dt = mybir.dt
AF = mybir.ActivationFunctionType
OP = mybir.AluOpType

_CACHE = {}


# ----------------------------------------------------------------------------
# host-side helpers
# ----------------------------------------------------------------------------

def _rope_cos_sin(num_tokens):
    base = 150000.0
    scaling = 32.0
    init_ctx = 4096.0
    ntk_alpha = 1.0
    ntk_beta = 32.0
    d_half = HD / 2
    freq = base ** (np.arange(0, HD, 2, dtype=np.float32) / HD)
    concentration = 0.1 * math.log(scaling) + 1.0
    low = d_half * math.log(init_ctx / (ntk_beta * 2 * math.pi)) / math.log(base)
    high = d_half * math.log(init_ctx / (ntk_alpha * 2 * math.pi)) / math.log(base)
    interpolation = 1.0 / (scaling * freq)
    extrapolation = 1.0 / freq
    ramp = (np.arange(int(d_half), dtype=np.float32) - low) / (high - low)
    m = 1.0 - np.clip(ramp, 0.0, 1.0)
    inv_freq = interpolation * (1.0 - m) + extrapolation * m
    t = np.arange(num_tokens, dtype=np.float32)
    freqs = t[:, None] * inv_freq[None, :]
    cos = (np.cos(freqs) * concentration).astype(np.float32)
    sin = (np.sin(freqs) * concentration).astype(np.float32)
    return cos, sin  # [T, 32]


def _rope_tables():
    cos, sin = _rope_cos_sin(T)  # [1024, 32]
    cos64 = np.concatenate([cos, cos], axis=1)             # [1024, 64]
    sin64 = np.concatenate([-sin, sin], axis=1)            # signed for halves

    def tok_major(a):  # [1024, 64] -> [128, 8*64]
        return a.reshape(MT, P, HD).transpose(1, 0, 2).reshape(P, MT * HD)

    return (
        tok_major(cos64 * SM_SCALE),
        tok_major(sin64 * SM_SCALE),
        tok_major(cos64),
        tok_major(sin64),
    )


def _prep_core_inputs(core, x, norm_scale, qkv_w, qkv_b, out_w, sinks):
    """Build the per-core input map (all numpy, layouts per module docstring)."""
    q_end = NH * HD
    k_end = q_end + NKV * HD

    # rows of qkv_w for this core: 8 q heads + 1 v head + 1 k head = 640 rows
    qrows = np.arange(core * HPC * HD, (core + 1) * HPC * HD)
    krows = np.arange(q_end + core * HD, q_end + (core + 1) * HD)
    vrows = np.arange(k_end + core * HD, k_end + (core + 1) * HD)
    rows = np.concatenate([qrows, vrows, krows])  # [640]: q | v | k

    wshard = (qkv_w[rows, :] * norm_scale[None, :]).astype(np.float32)
    bshard = qkv_b[rows].astype(np.float32)  # [640]

    def part_major(a2d, ncols):  # [2880, ncols] -> [128, 23, ncols] padded
        out = np.zeros((KT * P, ncols), dtype=np.float32)
        out[:HIDDEN] = a2d
        return out.reshape(KT, P, ncols).transpose(1, 0, 2).astype(BF16).copy()

    wk = part_major(wshard.T, NC)                # [128, 23, 640]
    xt = part_major(x.T.astype(np.float32), T)   # [128, 23, 1024]

    # out_w shard: columns for this core's heads -> [512 hd, 2880 H]
    cols = np.arange(core * HPC * HD, (core + 1) * HPC * HD)
    wo = out_w[:, cols].T.astype(np.float32)
    wout = wo.reshape(4, P, HIDDEN).transpose(1, 0, 2).astype(BF16).copy()

    # const blob: bias | mask01 | esink | rope tables
    j = np.arange(P)[:, None]
    i = np.arange(P)[None, :]
    mask_prev = (j > i).astype(np.float32)
    mask_self = (j <= i).astype(np.float32)
    esink = np.exp(sinks[core * HPC:(core + 1) * HPC].astype(np.float64))
    esink = np.broadcast_to(esink.astype(np.float32), (P, HPC))
    cosq, sinq, cosk, sink_t = _rope_tables()
    cblob = np.concatenate([
        np.broadcast_to(bshard, (P, NC)),
        mask_prev, mask_self, esink, cosq, sinq, cosk, sink_t,
    ], axis=1).astype(BF16)
    assert cblob.shape == (P, CB)

    return {"xt": xt, "wk": wk, "wout": wout, "cblob": cblob.copy()}


# ----------------------------------------------------------------------------
# device kernel (Tile)
# ----------------------------------------------------------------------------

def build_nc():
    nc = bacc.Bacc("TRN2", target_bir_lowering=False, debug=False)

    xt_d = nc.dram_tensor("xt", [P, KT, T], dt.bfloat16, kind="ExternalInput").ap()
    wk_d = nc.dram_tensor("wk", [P, KT, NC], dt.bfloat16, kind="ExternalInput").ap()
    wout_d = nc.dram_tensor("wout", [P, 4, HIDDEN], dt.bfloat16,
                            kind="ExternalInput").ap()
    cblob_d = nc.dram_tensor("cblob", [P, CB], dt.bfloat16,
                             kind="ExternalInput").ap()
    y_d = nc.dram_tensor("y", [T, HIDDEN], dt.bfloat16, kind="ExternalOutput").ap()

    KGRP = [(0, 6), (6, 12), (12, 18), (18, KT)]  # xt/wk DMA chase groups

    def bcast_mid(ap2d, n):
        """[P, F] -> [P, n, F] with a 0-step middle dim (free broadcast)."""
        return bass.AP(tensor=ap2d.tensor, offset=ap2d.offset,
                       ap=[ap2d.ap[0], [0, n]] + list(ap2d.ap[1:]))

    with tile.TileContext(nc) as tc:
        with (
            tc.tile_pool(name="const", bufs=1) as const,
            tc.tile_pool(name="res", bufs=1) as res,
            tc.tile_pool(name="xsqp", bufs=3) as xsqp,
            tc.tile_pool(name="qkvp", bufs=3) as qkvp,
            tc.tile_pool(name="ropep", bufs=2) as ropep,
            tc.tile_pool(name="qrap", bufs=3) as qrap,
            tc.tile_pool(name="ptp", bufs=4) as ptp,
            tc.tile_pool(name="anormp", bufs=3) as anormp,
            tc.tile_pool(name="atp", bufs=6) as atp,
            tc.tile_pool(name="smallp", bufs=4) as smallp,
            tc.tile_pool(name="ysbp", bufs=2) as ysbp,
            tc.tile_pool(name="pmix", bufs=2, space="PSUM") as pmix,
            tc.tile_pool(name="pq", bufs=4, space="PSUM") as pq,
            tc.tile_pool(name="pkvc", bufs=2, space="PSUM") as pkvc,
        ):
            # ---- DMA issue order: first chase groups, then consts, wout ----
            xt_sb = res.tile([P, KT, T], dt.bfloat16, tag="xt", name="xt")
            wk_sb = res.tile([P, KT, NC], dt.bfloat16, tag="wk", name="wk")
            for g0, g1 in KGRP[:2]:
                nc.sync.dma_start(out=wk_sb[:, g0:g1, :], in_=wk_d[:, g0:g1, :])
                nc.sync.dma_start(out=xt_sb[:, g0:g1, :], in_=xt_d[:, g0:g1, :])
            cb = const.tile([P, CB], dt.bfloat16, tag="cb", name="cb")
            nc.sync.dma_start(out=cb, in_=cblob_d)
            for g0, g1 in KGRP[2:]:
                nc.sync.dma_start(out=wk_sb[:, g0:g1, :], in_=wk_d[:, g0:g1, :])
                nc.sync.dma_start(out=xt_sb[:, g0:g1, :], in_=xt_d[:, g0:g1, :])
            wout_sb = res.tile([P, 4, HIDDEN], dt.bfloat16, tag="wout",
                               name="wout")
            nc.sync.dma_start(out=wout_sb, in_=wout_d)

            # const blob views
            bias_sb = cb[:, 0:NC]
            mask_prev = cb[:, NC:NC + P]
            mask_self = cb[:, NC + P:NC + 2 * P]
            esink_sb = cb[:, NC + 2 * P:NC + 2 * P + HPC]
            tb0 = NC + 2 * P + HPC
            tabs = {}
            for idx, nm in enumerate(("cosq", "sinq", "cosk", "sink_t")):
                tabs[nm] = cb[:, tb0 + idx * MT * HD:tb0 + (idx + 1) * MT * HD] \
                    .rearrange("p (t d) -> p t d", t=MT)

            # ---- constants ----
            ones_b = const.tile([P, 1], dt.bfloat16, tag="ones", name="ones")
            nc.vector.memset(ones_b, 1.0)
            ident_b = const.tile([P, P], dt.bfloat16, tag="identb",
                                 name="identb")
            make_identity(nc, ident_b)
            ident_f = const.tile([1, 1], dt.float32, tag="identf", name="identf")
            nc.vector.memset(ident_f, 1.0)
            zbias = const.tile([P, 1], dt.float32, tag="zbias", name="zbias")
            nc.vector.memset(zbias, 0.0)
            eps_t = const.tile([P, 1], dt.float32, tag="eps", name="eps")
            nc.vector.memset(eps_t, 1e-5)

            # ---- wave A: QKV chains for token tiles 0-3 + ssq chase ----
            pq_t = {}
            for t in range(4):
                pq_t[t] = pq.tile([P, NQ], dt.float32, tag="pq", name=f"pq{t}")
            pssq = [pmix.tile([1, 512], dt.float32, tag="mix", name=f"ssq{i}")
                    for i in range(2)]
            for ki in range(KT):
                xsq = xsqp.tile([P, T], dt.bfloat16, tag="xsq", name="xsq")
                nc.vector.tensor_tensor(xsq, xt_sb[:, ki, :], xt_sb[:, ki, :],
                                        OP.mult)
                for i in range(2):
                    nc.tensor.matmul(pssq[i], ones_b,
                                     xsq[:, i * 512:(i + 1) * 512],
                                     start=(ki == 0), stop=(ki == KT - 1))
                for t in range(4):
                    nc.tensor.matmul(pq_t[t], xt_sb[:, ki, t * P:(t + 1) * P],
                                     wk_sb[:, ki, 0:NQ],
                                     start=(ki == 0), stop=(ki == KT - 1))

            # ---- ssq -> rsq8 [128 tok, 8 tile] ----
            ssq_sb = res.tile([1, T], dt.float32, tag="ssq", name="ssq")
            nc.vector.tensor_copy(ssq_sb[:, 0:512], pssq[0])
            nc.scalar.copy(ssq_sb[:, 512:1024], pssq[1])
            ssqT = pmix.tile([P, MT], dt.float32, tag="mix", name="ssqT")
            for t in range(MT):
                nc.tensor.transpose(ssqT[:, t:t + 1],
                                    ssq_sb[0:1, t * P:(t + 1) * P],
                                    ident_f)
            lnm = res.tile([P, MT], dt.float32, tag="lnm", name="lnm")
            nc.scalar.activation(lnm, ssqT, AF.Ln, bias=eps_t, scale=1.0 / HIDDEN)
            rsq8 = res.tile([P, MT], dt.float32, tag="rsq8", name="rsq8")
            nc.scalar.activation(rsq8, lnm, AF.Exp, bias=zbias, scale=-0.5)

            # ---- per-token-tile post-QKV: drain, rope, transposes ----
            krope = res.tile([HD, T], dt.bfloat16, tag="krope", name="krope")
            vtok = [res.tile([P, AW], dt.bfloat16, tag=f"vtok{b}",
                             name=f"vtok{b}")
                    for b in range(MT)]
            qra = {}

            def drain_rope(t, pq_tile, pkv_ap):
                """psum -> qkv_sb (fused rsq scale + bias); rope; PE transposes."""
                rsq = rsq8[:, t:t + 1]
                qkv_sb = qkvp.tile([P, NC], dt.bfloat16, tag="qkv", name="qkv")
                nc.vector.scalar_tensor_tensor(qkv_sb[:, 0:NQ], pq_tile, rsq,
                                               bias_sb[:, 0:NQ], OP.mult,
                                               OP.add)
                nc.vector.scalar_tensor_tensor(qkv_sb[:, NQ:NC], pkv_ap, rsq,
                                               bias_sb[:, NQ:NC], OP.mult,
                                               OP.add)
                # v -> vtok (no rope)
                nc.gpsimd.tensor_copy(vtok[t][:, 0:HD], qkv_sb[:, NQ:NQ + HD])
                nc.gpsimd.memset(vtok[t][:, HD:HD + 1], 1.0)
                # rope q: view [P, 8, 64]
                q3 = qkv_sb[:, 0:NQ].rearrange("p (h d) -> p h d", h=HPC)
                m1 = ropep.tile([P, HPC, HD], dt.bfloat16, tag="m1", name="m1")
                m2 = ropep.tile([P, HPC, HD], dt.bfloat16, tag="m2", name="m2")
                cq = tabs["cosq"][:, t, :]
                sq = tabs["sinq"][:, t, :]
                nc.vector.tensor_tensor(m1, q3, bcast_mid(cq, HPC), OP.mult)
                nc.vector.tensor_tensor(m2[:, :, 0:32], q3[:, :, 32:64],
                                        bcast_mid(sq[:, 0:32], HPC), OP.mult)
                nc.vector.tensor_tensor(m2[:, :, 32:64], q3[:, :, 0:32],
                                        bcast_mid(sq[:, 32:64], HPC), OP.mult)
                qrot = ropep.tile([P, HPC, HD], dt.bfloat16, tag="qrot",
                                  name="qrot")
                nc.vector.tensor_tensor(qrot, m1, m2, OP.add)
                # rope k: [P, 64] at cols 576:640
                k3 = qkv_sb[:, NQ + HD:NC]
                ck = tabs["cosk"][:, t, :]
                sk = tabs["sink_t"][:, t, :]
                km1 = ropep.tile([P, HD], dt.bfloat16, tag="km1", name="km1")
                km2 = ropep.tile([P, HD], dt.bfloat16, tag="km2", name="km2")
                nc.vector.tensor_tensor(km1, k3, ck, OP.mult)
                nc.vector.tensor_tensor(km2[:, 0:32], k3[:, 32:64],
                                        sk[:, 0:32], OP.mult)
                nc.vector.tensor_tensor(km2[:, 32:64], k3[:, 0:32],
                                        sk[:, 32:64], OP.mult)
                krot = ropep.tile([P, HD], dt.bfloat16, tag="krot", name="krot")
                nc.vector.tensor_tensor(krot, km1, km2, OP.add)
                # PE transposes: q -> [2h*64, 128 tok] pairs; k -> [64, 128]
                qr2 = qrot.rearrange("p h d -> p (h d)")
                ptq = pmix.tile([P, 4, P], dt.bfloat16, tag="mix", name="ptq")
                for hp in range(4):
                    nc.tensor.transpose(ptq[:, hp, :],
                                        qr2[:, hp * P:(hp + 1) * P], ident_b)
                ptk = pmix.tile([HD, P], dt.bfloat16, tag="mix", name="ptk")
                nc.tensor.transpose(ptk, krot, ident_b)
                qra_t = qrap.tile([HD, HPC, P], dt.bfloat16, tag="qra",
                                  name="qra")
                qv = qra_t.rearrange("p (a b) t -> p a b t", b=2)
                nc.vector.tensor_copy(qv[:, :, 0, :], ptq[0:HD, :, :])
                nc.vector.tensor_copy(qv[:, :, 1, :], ptq[HD:P, :, :])
                nc.scalar.copy(krope[:, t * P:(t + 1) * P], ptk)
                qra[t] = qra_t

            # ---- attention front half: scores, exp, mask for tile b ----
            att_tiles = {}

            def attn_scores(b):
                pts = []
                for kt, msk in ((b - 1, mask_prev), (b, mask_self)):
                    if kt < 0:
                        pts.append(None)
                        continue
                    pt_g = []
                    for g in range(2):
                        ps = pq.tile([P, 4, P], dt.float32, tag="pq",
                                     name="score")
                        nc.tensor.matmul(
                            ps, krope[:, kt * P:(kt + 1) * P],
                            qra[b][:, 4 * g:4 * g + 4, :],
                            start=True, stop=True)
                        pt = ptp.tile([P, 4, P], dt.bfloat16, tag="pt",
                                      name="pt")
                        nc.scalar.activation(pt, ps, AF.Exp, bias=zbias)
                        nc.vector.tensor_tensor(pt, pt, bcast_mid(msk, 4),
                                                OP.mult)
                        pt_g.append(pt)
                    pts.append(pt_g)
                return pts

            # ---- attention back half: AV, normalize, transpose ----
            def attn_av(b, pts):
                ptA, ptB = pts
                rec8 = smallp.tile([P, HPC], dt.float32, tag="rec8",
                                   name="rec8")
                att = []
                for g in range(2):
                    pg = pmix.tile([P, 4, AW], dt.float32, tag="mix", name="pg")
                    for j in range(4):
                        if b > 0:
                            nc.tensor.matmul(pg[:, j, :], ptA[g][:, j, :],
                                             vtok[b - 1], start=True,
                                             stop=False)
                            nc.tensor.matmul(pg[:, j, :], ptB[g][:, j, :],
                                             vtok[b], start=False, stop=True)
                        else:
                            nc.tensor.matmul(pg[:, j, :], ptB[g][:, j, :],
                                             vtok[b], start=True, stop=True)
                    g0 = 4 * g
                    nc.vector.tensor_tensor(rec8[:, g0:g0 + 4],
                                            pg[:, :, HD:HD + 1],
                                            esink_sb[:, g0:g0 + 4], OP.add)
                    nc.vector.reciprocal(rec8[:, g0:g0 + 4], rec8[:, g0:g0 + 4])
                    an = anormp.tile([P, 4, HD], dt.bfloat16, tag="anorm",
                                     name="anorm")
                    rec3 = bass.AP(tensor=rec8.tensor,
                                   offset=rec8[:, g0:g0 + 4].offset,
                                   ap=[rec8.ap[0], [1, 4], [0, HD]])
                    nc.vector.tensor_tensor(an, pg[:, :, 0:HD], rec3, OP.mult)
                    a2 = an.rearrange("p a b -> p (a b)")
                    for j in range(2):
                        pat = pmix.tile([P, P], dt.bfloat16, tag="mix",
                                        name="pat")
                        nc.tensor.transpose(pat, a2[:, j * P:(j + 1) * P],
                                            ident_b)
                        at = atp.tile([P, P], dt.bfloat16, tag="at", name="at")
                        if (g + j) % 2 == 0:
                            nc.vector.tensor_copy(at, pat)
                        else:
                            nc.scalar.copy(at, pat)
                        att.append(at)
                att_tiles[b] = att

            # ---- out projection for tile b (runs one tile behind) ----
            def outproj(b):
                att = att_tiles.pop(b)
                ysb = ysbp.tile([P, HIDDEN], dt.bfloat16, tag="ysb", name="ysb")
                for w0 in range(0, HIDDEN // YC, 2):
                    pys = [pq.tile([P, YC], dt.float32, tag="pq", name="py")
                           for _ in range(2)]
                    for kk in range(4):
                        for ci in range(2):
                            o0 = (w0 + ci) * YC
                            nc.tensor.matmul(pys[ci], att[kk],
                                             wout_sb[:, kk, o0:o0 + YC],
                                             start=(kk == 0), stop=(kk == 3))
                    for ci in range(2):
                        o0 = (w0 + ci) * YC
                        nc.scalar.activation(ysb[:, o0:o0 + YC], pys[ci],
                                             AF.Copy)
                nc.sync.dma_start(out=y_d[b * P:(b + 1) * P, :], in_=ysb)

            # kv chains: short ki-inner accumulations (one pending group
            # per 2KB psum region), run after xt/wk are resident
            def kv_chain(t):
                kvt = pkvc.tile([P, NQ], dt.float32, tag="kvc", name=f"kv{t}")
                for ki in range(KT):
                    nc.tensor.matmul(kvt[:, 0:2 * HD],
                                     xt_sb[:, ki, t * P:(t + 1) * P],
                                     wk_sb[:, ki, NQ:NC],
                                     start=(ki == 0), stop=(ki == KT - 1))
                return kvt

            def q_chain(t):
                pq_tile = pq.tile([P, NQ], dt.float32, tag="pq", name=f"pq{t}")
                for ki in range(KT):
                    nc.tensor.matmul(pq_tile, xt_sb[:, ki, t * P:(t + 1) * P],
                                     wk_sb[:, ki, 0:NQ],
                                     start=(ki == 0), stop=(ki == KT - 1))
                return pq_tile

            # wave A drains (tiles 0-3) interleaved with their kv chains
            kv0 = kv_chain(0)
            kv1 = kv_chain(1)
            drain_rope(0, pq_t[0], kv0[:, 0:2 * HD])
            kv2 = kv_chain(2)
            drain_rope(1, pq_t[1], kv1[:, 0:2 * HD])
            kv3 = kv_chain(3)
            drain_rope(2, pq_t[2], kv2[:, 0:2 * HD])
            drain_rope(3, pq_t[3], kv3[:, 0:2 * HD])

            # software-pipelined steady state: for each round, emit scores(b),
            # then AV(b-1) / outproj(b-2) as PE filler while exp/mask cook;
            # wave-B qkv chains interleave
            pend = {}
            pend[0] = attn_scores(0)
            for t in range(4, MT):
                b = t - 3           # front tile this round: 1..4
                pq_tile = q_chain(t)
                kvt = kv_chain(t)
                drain_rope(t, pq_tile, kvt[:, 0:2 * HD])
                attn_av(b - 1, pend.pop(b - 1))
                pend[b] = attn_scores(b)
                if b >= 2:
                    outproj(b - 2)
            for b in range(5, MT):
                attn_av(b - 1, pend.pop(b - 1))
                pend[b] = attn_scores(b)
                outproj(b - 2)
            attn_av(MT - 1, pend.pop(MT - 1))
            outproj(MT - 2)
            outproj(MT - 1)

    nc.compile()
    return nc


# ----------------------------------------------------------------------------
# public entry
# ----------------------------------------------------------------------------

LAST_RESULTS = None


def kernel(x, norm_scale, qkv_w, qkv_b, out_w, out_b, sinks):
    global LAST_RESULTS
    x = np.asarray(x, dtype=np.float32)
    norm_scale = np.asarray(norm_scale, dtype=np.float32)
    qkv_w = np.asarray(qkv_w, dtype=np.float32)
    qkv_b = np.asarray(qkv_b, dtype=np.float32)
    out_w = np.asarray(out_w, dtype=np.float32)
    out_b = np.asarray(out_b, dtype=np.float32)
    sinks = np.asarray(sinks, dtype=np.float32)

    if "nc" not in _CACHE:
        _CACHE["nc"] = build_nc()
    nc = _CACHE["nc"]

    in_maps = [
        _prep_core_inputs(c, x, norm_scale, qkv_w, qkv_b, out_w, sinks)
        for c in range(NCORES)
    ]
    import os
    tmpdir = os.environ.get("BASS_TMPDIR") or None
    res = run_bass_kernel_spmd(nc, in_maps, core_ids=list(range(NCORES)),
                               tmpdir=tmpdir)
    LAST_RESULTS = res
    y = np.zeros((T, HIDDEN), dtype=np.float64)
    for c in range(NCORES):
        y += res.results[c]["y"].astype(np.float64)
    y += out_b.astype(np.float64)[None, :]
    return y.astype(np.float32)
